# revision 15
# baseline (speedup 1.0000x reference)
"""Trainium2 Bass/Tile kernel: single-head attention (B=8, S=2048, E=1024, DQ=DV=128).

Data-parallel over the batch: one batch element per NeuronCore (8 cores), no
collectives. v7 layout:

  * query/key stream in as fp8 e4m3 (weights pre-scaled x64); the q/k
    projections run in DoubleRow perf mode (2 fp8 MACs/cell/cycle, K=256 per
    pass) - halves projection PE time.
  * value/Wv stay bf16: the attention output is a weighted MEAN of v, so
    per-element v quantization error passes to the output 1:1 (signal and
    noise average down together) - fp8 v measurably blows the error budget.
  * ALL input DMA is issued from the sync (SP) ring. Issuing DMAs from
    nc.scalar head-of-line blocks the exp chain: each dma_start costs
    ~0.65us of descriptor-gen on its sequencer, and with only 8 DMAHW
    completion lanes the 9th+ transfer's descgen stalls until an earlier
    transfer's data fully lands - measured to push the first exp to ~24us.
  * Transfer count minimized: wq rides in xq0's transfer, wk in xk0's,
    (padb|tri|bq|bk) are one packed bf16 tensor, (wv|bvb) one tensor.
  * Activation streams are sequence-blocked AND host-pre-arranged so each
    granule is contiguous per partition line. Chunk pairs (dim1) double as
    the DoubleRow K-pair axis.
  * Warm-up matmuls on a zeroed tile run during the DMA-fill window (plus
    small filler groups before each granule-gated piece) so HAM reaches and
    keeps K=8/8.
  * scoresT = kT_blk.T @ qT in [keys, queries] layout; exp on ACT with the
    pad mask as a per-partition bias; in-block causal mask via a DVE
    triangular multiply. AV: stationary = attnT block, moving = v_aug with a
    ones column that makes the AV matmul emit softmax row sums for free.
  * Output ships the un-normalized numerator + row sum ([S, DV+1] bf16);
    the final divide happens on host in fp32.
  * Schedule interleaves proj/scores/vnat/AV so the PE never head-of-line
    blocks on the exp chain (ps_sc double buffering paces scores to exp).
"""

import numpy as np
import ml_dtypes
from contextlib import ExitStack

B, S, E, DQ, DV = 8, 2048, 1024, 128, 128
EC = E // 128    # contraction chunks
EC2 = EC // 2    # DoubleRow chunk pairs
SC = S // 128    # sequence chunks
QB = 512         # matmul moving-dim block / granule seq width
NG = S // QB     # granules per activation stream
WSCALE = 64.0    # fp8 weight pre-scale for Wq/Wk
RSQRT_DQ = 1.0 / float(np.sqrt(DQ))
NEG = np.float32(-1e9)
_BF16 = ml_dtypes.bfloat16
_E4M3 = ml_dtypes.float8_e4m3

_prog = None


def _build_program():
    import concourse.bacc as bacc
    import concourse.mybir as mybir
    import concourse.tile as tile

    f32 = mybir.dt.float32
    bf16 = mybir.dt.bfloat16
    f8e4 = mybir.dt.float8e4
    AF = mybir.ActivationFunctionType
    ALU = mybir.AluOpType
    DR = mybir.MatmulPerfMode.DoubleRow

    nc = bacc.Bacc("TRN2", target_bir_lowering=False, debug=False)

    # misc = [padb(16) | tri(128) | bq f32-bits(2) | bk f32-bits(2)]
    d_misc = nc.dram_tensor("misc", [128, SC + 128 + 4], bf16,
                            kind="ExternalInput").ap()
    # wqx = [wq(128) | xq granule 0 cols 0:256] per chunk (+ xq0b rest)
    d_wqx = nc.dram_tensor("wqx", [128, EC, 128 + QB // 2], f8e4,
                           kind="ExternalInput").ap()
    d_wkx = nc.dram_tensor("wkx", [128, EC, 128 + QB // 2], f8e4,
                           kind="ExternalInput").ap()
    d_qxb = nc.dram_tensor("qxb", [128, EC, QB // 2], f8e4,
                           kind="ExternalInput").ap()
    d_kxb = nc.dram_tensor("kxb", [128, EC, QB // 2], f8e4,
                           kind="ExternalInput").ap()
    d_qx = nc.dram_tensor("qx", [NG - 1, 128, EC, QB], f8e4,
                          kind="ExternalInput").ap()
    d_kx = nc.dram_tensor("kx", [NG - 1, 128, EC, QB], f8e4,
                          kind="ExternalInput").ap()
    d_wvb = nc.dram_tensor("wvb", [128, EC * 128 + DV], bf16,
                           kind="ExternalInput").ap()
    d_vx = nc.dram_tensor("vx", [NG, 128, EC, QB], bf16,
                          kind="ExternalInput").ap()
    # out groups of rows: 4,4,4,2,1,1 (small final groups shorten the
    # post-row15 DMA tail); partition-major [128, row, DV+1] layout keeps
    # each group's per-partition bytes contiguous in HBM
    d_out = nc.dram_tensor("out", [128, SC, DV + 1], bf16,
                           kind="ExternalOutput").ap()

    with tile.TileContext(nc) as tc, ExitStack() as ctx:
        consts = ctx.enter_context(tc.tile_pool(name="consts", bufs=1))
        xq_p = ctx.enter_context(tc.tile_pool(name="xq", bufs=NG - 1))
        xk_p = ctx.enter_context(tc.tile_pool(name="xk", bufs=NG - 1))
        xv_p = ctx.enter_context(tc.tile_pool(name="xv", bufs=NG))
        proj_p = ctx.enter_context(tc.tile_pool(name="proj", bufs=1))
        attn_p = ctx.enter_context(tc.tile_pool(name="attn", bufs=1))
        out_p = ctx.enter_context(tc.tile_pool(name="outp", bufs=6))
        # PSUM budget: proj/vnat 2 banks + scores 4 + AV 2 = 8
        ps_main = ctx.enter_context(tc.tile_pool(name="ps_main", bufs=2, space="PSUM"))
        ps_sc = ctx.enter_context(tc.tile_pool(name="ps_sc", bufs=2, space="PSUM"))
        ps_av = ctx.enter_context(tc.tile_pool(name="ps_av", bufs=2, space="PSUM"))

        # ---- input DMA issue: ONE ring (sync/SP), arrival-priority order --
        wqx = consts.tile([128, EC, 128 + QB // 2], f8e4, tag="wqx")
        nc.sync.dma_start(wqx[:, :, :], d_wqx)
        wkx = consts.tile([128, EC, 128 + QB // 2], f8e4, tag="wkx")
        nc.sync.dma_start(wkx[:, :, :], d_wkx)
        misc = consts.tile([128, SC + 128 + 4], bf16, tag="misc")
        nc.sync.dma_start(misc[:, :], d_misc)
        padb = misc[:, 0:SC]
        tri = misc[:, SC:SC + 128]
        bq = misc[:, SC + 128:SC + 130].bitcast(f32)
        bk = misc[:, SC + 130:SC + 132].bitcast(f32)
        qxb = consts.tile([128, EC, QB // 2], f8e4, tag="qxb")
        nc.sync.dma_start(qxb[:, :, :], d_qxb)
        kxb = consts.tile([128, EC, QB // 2], f8e4, tag="kxb")
        nc.sync.dma_start(kxb[:, :, :], d_kxb)

        xq = [(wqx, 128)]
        xk = [(wkx, 128)]
        for g in range(1, NG):
            tq = xq_p.tile([128, EC, QB], f8e4, tag="xq", name=f"xq{g}")
            nc.sync.dma_start(tq[:, :, :], d_qx[g - 1])
            xq.append((tq, 0))
            tk = xk_p.tile([128, EC, QB], f8e4, tag="xk", name=f"xk{g}")
            nc.sync.dma_start(tk[:, :, :], d_kx[g - 1])
            xk.append((tk, 0))

        wvb = consts.tile([128, EC * 128 + DV], bf16, tag="wvb")
        nc.sync.dma_start(wvb[:, :], d_wvb)
        bvb = wvb[:, EC * 128:EC * 128 + DV]

        xv = []
        for g in range(NG):
            t = xv_p.tile([128, EC, QB], bf16, tag="xv", name=f"xv{g}")
            nc.sync.dma_start(t[:, :, :], d_vx[g])
            xv.append(t)

        # ---- warmup: exp LUT load + PE HAM ramp during the DMA window ----
        warm = consts.tile([128, QB], bf16, tag="warm")
        nc.vector.memset(warm[:, :], 0.0)
        wo = consts.tile([128, 1], f32, tag="warmo")
        nc.scalar.activation(wo[:, :], warm[:, 0:1], AF.Exp)
        wps = ps_main.tile([128, QB], f32, tag="ps", name="warmps")
        for i in range(9):
            nc.tensor.matmul(wps[:, :], warm[:, 0:128], warm[:, :],
                             start=(i == 0), stop=(i == 8))

        qT = proj_p.tile([128, S], bf16, tag="qT")
        kT = proj_p.tile([128, S], bf16, tag="kT")

        def proj_piece(dst, w, bias, p):
            # dst[:, p*QB:(p+1)*QB] = (64W).T @ x_granule(p) + 64b
            # DoubleRow: K=256 per pass via adjacent chunk pairs (e4m3).
            # Piece 0's granule is split across two transfers (wqx carries
            # cols 0:256, qxb the rest) so the first matmul starts sooner.
            ps = ps_main.tile([128, QB], f32, tag="ps")
            if p == 0:
                xa = wqx if dst is qT else wkx
                xb = qxb if dst is qT else kxb
                H = QB // 2
                for c in range(EC2):
                    nc.tensor.matmul(ps[:, 0:H], w[:, 2 * c:2 * c + 2, 0:128],
                                     xa[:, 2 * c:2 * c + 2, 128:128 + H],
                                     start=(c == 0), stop=(c == EC2 - 1),
                                     perf_mode=DR)
                for c in range(EC2):
                    nc.tensor.matmul(ps[:, H:QB], w[:, 2 * c:2 * c + 2, 0:128],
                                     xb[:, 2 * c:2 * c + 2, :],
                                     start=(c == 0), stop=(c == EC2 - 1),
                                     perf_mode=DR)
            else:
                xt, off = (xq if dst is qT else xk)[p]
                for c in range(EC2):
                    nc.tensor.matmul(ps[:, :], w[:, 2 * c:2 * c + 2, 0:128],
                                     xt[:, 2 * c:2 * c + 2, off:off + QB],
                                     start=(c == 0), stop=(c == EC2 - 1),
                                     perf_mode=DR)
            nc.vector.tensor_scalar(dst[:, p * QB:(p + 1) * QB], ps[:, :],
                                    bias, None, ALU.add)

        attnT = [attn_p.tile([128, S - j * 128], bf16, tag=f"attnT{j}",
                             name=f"attnT{j}")
                 for j in range(SC)]
        vaug = [attn_p.tile([128, DV + 1], bf16, tag=f"vaug{j}",
                            name=f"vaug{j}")
                for j in range(SC)]
        for j in range(SC):
            nc.vector.memset(vaug[j][:, DV:DV + 1], 1.0)

        def scores_win(j, a0, a1):
            # scoresT[j], abs q cols [a0, a1) -> exp -> attnT[j] slice (bf16)
            n = a1 - a0
            ps = ps_sc.tile([128, n], f32, tag="ps_sc")
            for q0 in range(a0, a1, QB):
                m = min(QB, a1 - q0)
                nc.tensor.matmul(ps[:, q0 - a0:q0 - a0 + m],
                                 kT[:, j * 128:(j + 1) * 128],
                                 qT[:, q0:q0 + m], start=True, stop=True)
            nc.scalar.activation(attnT[j][:, a0 - j * 128:a1 - j * 128],
                                 ps[:, :], AF.Exp,
                                 bias=padb[:, j:j + 1],
                                 scale=RSQRT_DQ / (WSCALE * WSCALE))

        def tri_mask(j):
            # in-block causal mask on the diagonal block (keep k <= q)
            nc.vector.tensor_mul(attnT[j][:, 0:128], attnT[j][:, 0:128],
                                 tri)

        def vnat_pair(jp):
            # v natural [keys, DV] for chunks 2jp, 2jp+1: stationary = value
            # seq-slice, moving = Wv chunk; + bias along DV via bvb.
            js = (2 * jp, 2 * jp + 1)
            pss = [ps_main.tile([128, 128], f32, tag="ps", name=f"psv{j}")
                   for j in js]
            for c in range(EC):
                for ji, j in enumerate(js):
                    g, k0 = j // 4, (j % 4) * 128
                    nc.tensor.matmul(pss[ji][:, :], xv[g][:, c, k0:k0 + 128],
                                     wvb[:, c * 128:(c + 1) * 128],
                                     start=(c == 0), stop=(c == EC - 1))
            for ji, j in enumerate(js):
                nc.vector.tensor_add(vaug[j][:, 0:DV], pss[ji][:, :],
                                     bvb)

        OGRP = {}  # row -> (group start, group len)
        for g0, gl in ((0, 4), (4, 4), (8, 4), (12, 2), (14, 1), (15, 1)):
            for i in range(g0, g0 + gl):
                OGRP[i] = (g0, gl)
        ostage = [None]

        def out_row(i, ps):
            # stage output rows and DMA per group: fewer transfers (descgen
            # and DMAHW-lane churn on the sync ring was backpressuring the
            # AV chain through out-tile recycling)
            g0, gl = OGRP[i]
            if i == g0:
                ostage[0] = out_p.tile([128, 4, DV + 1], bf16, tag="ot",
                                       name=f"ost{g0}")
            nc.vector.tensor_copy(ostage[0][:, i - g0, :], ps[:, :])
            if i == g0 + gl - 1:
                nc.sync.dma_start(d_out[:, g0:g0 + gl, :],
                                  ostage[0][:, 0:gl, :])

        def av_row(i):
            ps = ps_av.tile([128, DV + 1], f32, tag="pso")
            for j in range(i + 1):
                nc.tensor.matmul(ps[:, :],
                                 attnT[j][:, (i - j) * 128:(i - j) * 128 + 128],
                                 vaug[j][:, :], start=(j == 0), stop=(j == i))
            out_row(i, ps)

        # ---- interleaved schedule (PE FIFO order == priority order) ----
        # Ordered to match granule arrival (xq g / xk g alternate, xv last).
        # Warm filler matmul groups sit just before the granule-gated pieces
        # so a late DMA can't idle the PE long enough for HAM to re-throttle.
        def filler(name, nmm=2):
            fps = ps_main.tile([128, QB], f32, tag="ps", name=name)
            for i in range(nmm):
                nc.tensor.matmul(fps[:, :], warm[:, 0:128], warm[:, :],
                                 start=(i == 0), stop=(i == nmm - 1))

        proj_piece(qT, wqx, bq, 0)
        filler("fill_k0", 2)
        proj_piece(kT, wkx, bk, 0)
        scores_win(0, 0, 512)
        filler("fill_q1", 2)
        proj_piece(qT, wqx, bq, 1)
        scores_win(0, 512, 1024)
        scores_win(1, 128, 1024)
        filler("fill_k1", 2)
        proj_piece(kT, wkx, bk, 1)
        scores_win(2, 256, 1024)
        scores_win(3, 384, 1024)
        filler("fill_q2", 2)
        proj_piece(qT, wqx, bq, 2)
        scores_win(4, 512, 1024)
        scores_win(5, 640, 1024)
        filler("fill_k2", 2)
        proj_piece(kT, wkx, bk, 2)
        scores_win(6, 768, 1024)
        scores_win(7, 896, 1024)
        proj_piece(qT, wqx, bq, 3)
        scores_win(0, 1024, 2048)
        tri_mask(0)
        proj_piece(kT, wkx, bk, 3)
        scores_win(1, 1024, 2048)
        tri_mask(1)
        scores_win(2, 1024, 2048)
        tri_mask(2)
        vnat_pair(0)
        scores_win(3, 1024, 2048)
        tri_mask(3)
        scores_win(4, 1024, 2048)
        tri_mask(4)
        vnat_pair(1)
        scores_win(5, 1024, 2048)
        tri_mask(5)
        scores_win(6, 1024, 2048)
        tri_mask(6)
        vnat_pair(2)
        scores_win(7, 1024, 2048)
        tri_mask(7)
        scores_win(8, 1024, 2048)
        tri_mask(8)
        vnat_pair(3)
        av_row(0)
        av_row(1)
        scores_win(9, 1152, 2048)
        tri_mask(9)
        av_row(2)
        av_row(3)
        scores_win(10, 1280, 2048)
        tri_mask(10)
        vnat_pair(4)
        av_row(4)
        av_row(5)
        scores_win(11, 1408, 2048)
        tri_mask(11)
        vnat_pair(5)
        av_row(6)
        av_row(7)
        scores_win(12, 1536, 2048)
        tri_mask(12)
        vnat_pair(6)
        av_row(8)
        av_row(9)
        scores_win(13, 1664, 2048)
        tri_mask(13)
        vnat_pair(7)
        # push the last score windows ahead of remaining AV so the exp
        # chain (ACT) finishes while the PE grinds AV matmuls
        scores_win(14, 1792, 2048)
        tri_mask(14)
        av_row(10)
        scores_win(15, 1920, 2048)
        tri_mask(15)
        av_row(11)
        # rows 13-15: accumulate j<=12 while the last exps are still in
        # flight, so the PE tail after exp15 is tiny.
        ps13 = ps_av.tile([128, DV + 1], f32, tag="pso", name="ps13")
        for j in range(0, 13):
            nc.tensor.matmul(ps13[:, :],
                             attnT[j][:, (13 - j) * 128:(13 - j) * 128 + 128],
                             vaug[j][:, :], start=(j == 0), stop=False)
        av_row(12)
        ps14 = ps_main.tile([128, DV + 1], f32, tag="ps", name="ps14")
        for j in range(0, 13):
            nc.tensor.matmul(ps14[:, :],
                             attnT[j][:, (14 - j) * 128:(14 - j) * 128 + 128],
                             vaug[j][:, :], start=(j == 0), stop=False)
        ps15 = ps_main.tile([128, DV + 1], f32, tag="ps", name="ps15")
        for j in range(0, 13):
            nc.tensor.matmul(ps15[:, :],
                             attnT[j][:, (15 - j) * 128:(15 - j) * 128 + 128],
                             vaug[j][:, :], start=(j == 0), stop=False)
        for i, psx in ((13, ps13), (14, ps14), (15, ps15)):
            for j in range(13, i + 1):
                nc.tensor.matmul(psx[:, :],
                                 attnT[j][:, (i - j) * 128:(i - j) * 128 + 128],
                                 vaug[j][:, :], start=False, stop=(j == i))
            out_row(i, psx)

    nc.compile()
    return nc


def _granulize(xT, dtype, width=QB):
    # [E, S] -> [S//width, 128, EC, width]: granule g holds all E rows for
    # seq slice [g*width,(g+1)*width), laid out so each partition line is
    # contiguous in HBM.
    return np.ascontiguousarray(
        xT.reshape(EC, 128, S // width, width).transpose(2, 1, 0, 3)
        .astype(dtype))


def _prep_inputs(pad_mask, query, key, value, Wq, bq, Wk, bk, Wv, bv):
    def wprep(w, dtype, scale):
        return np.ascontiguousarray(
            (np.asarray(w, np.float32) * scale).astype(dtype)
            .reshape(EC, 128, 128).transpose(1, 0, 2))

    wq = wprep(Wq, _E4M3, WSCALE)      # [128, EC, 128]
    wk = wprep(Wk, _E4M3, WSCALE)
    wv = wprep(Wv, _BF16, 1.0)
    wvb = np.concatenate(
        [wv.reshape(128, EC * 128),
         np.broadcast_to(np.asarray(bv, np.float32).astype(_BF16),
                         (128, DV))], axis=1)
    tri = np.triu(np.ones((128, 128), np.float32)).astype(_BF16)

    pad_mask = np.asarray(pad_mask)
    query = np.clip(np.asarray(query, np.float32), -15.0, 15.0)
    key = np.clip(np.asarray(key, np.float32), -15.0, 15.0)
    value = np.asarray(value, np.float32)
    bq64 = np.asarray(bq, np.float32).reshape(128, 1) * WSCALE
    bk64 = np.asarray(bk, np.float32).reshape(128, 1) * WSCALE
    in_maps = []
    for b in range(B):
        padb = np.where(pad_mask[b], NEG, np.float32(0.0)).reshape(SC, 128).T
        misc = np.concatenate(
            [padb.astype(_BF16).view(np.uint16), tri.view(np.uint16),
             bq64.view(np.uint16), bk64.view(np.uint16)],
            axis=1).view(_BF16)
        qg = _granulize(query[b].T, _E4M3)
        kg = _granulize(key[b].T, _E4M3)
        H = QB // 2
        in_maps.append({
            "misc": np.ascontiguousarray(misc),
            "wqx": np.ascontiguousarray(
                np.concatenate([wq, qg[0][:, :, 0:H]], axis=2)),
            "wkx": np.ascontiguousarray(
                np.concatenate([wk, kg[0][:, :, 0:H]], axis=2)),
            "qxb": np.ascontiguousarray(qg[0][:, :, H:]),
            "kxb": np.ascontiguousarray(kg[0][:, :, H:]),
            "qx": np.ascontiguousarray(qg[1:]),
            "kx": np.ascontiguousarray(kg[1:]),
            "wvb": np.ascontiguousarray(wvb),
            "vx": _granulize(value[b].T, _BF16),
        })
    return in_maps


def _run(in_maps, trace=False, **kwargs):
    global _prog
    from concourse.bass_utils import run_bass_kernel_spmd
    if _prog is None:
        _prog = _build_program()
    return run_bass_kernel_spmd(_prog, in_maps, list(range(B)), trace=trace,
                                **kwargs)


def _unstage(arr):
    # [128, SC, DV+1] -> [S, DV+1]: seq = row*128 + p
    return np.ascontiguousarray(
        np.transpose(arr, (1, 0, 2)).reshape(S, DV + 1))


def _finish(raw):
    # raw: [B, S, DV+1] f32 (numerator | row-sum); normalize on host
    return np.ascontiguousarray(
        (raw[:, :, :DV] / raw[:, :, DV:DV + 1]).astype(np.float32))


def kernel(pad_mask, query, key, value, Wq, bq, Wk, bk, Wv, bv):
    in_maps = _prep_inputs(pad_mask, query, key, value, Wq, bq, Wk, bk, Wv, bv)
    res = _run(in_maps)
    raw = np.stack([_unstage(np.asarray(res.results[i]["out"]))
                    for i in range(B)])
    return _finish(raw.astype(np.float32))


# revision 16
# speedup vs baseline: 1.0246x; 1.0246x over previous
"""Trainium2 Bass/Tile kernel: single-head attention (B=8, S=2048, E=1024, DQ=DV=128).

Data-parallel over the batch: one batch element per NeuronCore (8 cores), no
collectives. v7 layout:

  * query/key stream in as fp8 e4m3 (weights pre-scaled x64); the q/k
    projections run in DoubleRow perf mode (2 fp8 MACs/cell/cycle, K=256 per
    pass) - halves projection PE time.
  * value/Wv stay bf16: the attention output is a weighted MEAN of v, so
    per-element v quantization error passes to the output 1:1 (signal and
    noise average down together) - fp8 v measurably blows the error budget.
  * ALL input DMA is issued from the sync (SP) ring. Issuing DMAs from
    nc.scalar head-of-line blocks the exp chain: each dma_start costs
    ~0.65us of descriptor-gen on its sequencer, and with only 8 DMAHW
    completion lanes the 9th+ transfer's descgen stalls until an earlier
    transfer's data fully lands - measured to push the first exp to ~24us.
  * Transfer count minimized: wq rides in xq0's transfer, wk in xk0's,
    (padb|tri|bq|bk) are one packed bf16 tensor, (wv|bvb) one tensor.
  * Activation streams are sequence-blocked AND host-pre-arranged so each
    granule is contiguous per partition line. Chunk pairs (dim1) double as
    the DoubleRow K-pair axis.
  * Warm-up matmuls on a zeroed tile run during the DMA-fill window (plus
    small filler groups before each granule-gated piece) so HAM reaches and
    keeps K=8/8.
  * scoresT = kT_blk.T @ qT in [keys, queries] layout; exp on ACT with the
    pad mask as a per-partition bias; in-block causal mask via a DVE
    triangular multiply. AV: stationary = attnT block, moving = v_aug with a
    ones column that makes the AV matmul emit softmax row sums for free.
  * Output ships the un-normalized numerator + row sum ([S, DV+1] bf16);
    the final divide happens on host in fp32.
  * Schedule interleaves proj/scores/vnat/AV so the PE never head-of-line
    blocks on the exp chain (ps_sc double buffering paces scores to exp).
"""

import numpy as np
import ml_dtypes
from contextlib import ExitStack

B, S, E, DQ, DV = 8, 2048, 1024, 128, 128
EC = E // 128    # contraction chunks
EC2 = EC // 2    # DoubleRow chunk pairs
SC = S // 128    # sequence chunks
QB = 512         # matmul moving-dim block / granule seq width
NG = S // QB     # granules per activation stream
WSCALE = 64.0    # fp8 weight pre-scale for Wq/Wk
RSQRT_DQ = 1.0 / float(np.sqrt(DQ))
NEG = np.float32(-1e9)
_BF16 = ml_dtypes.bfloat16
_E4M3 = ml_dtypes.float8_e4m3

_prog = None


def _build_program():
    import concourse.bacc as bacc
    import concourse.mybir as mybir
    import concourse.tile as tile

    f32 = mybir.dt.float32
    bf16 = mybir.dt.bfloat16
    f8e4 = mybir.dt.float8e4
    AF = mybir.ActivationFunctionType
    ALU = mybir.AluOpType
    DR = mybir.MatmulPerfMode.DoubleRow

    nc = bacc.Bacc("TRN2", target_bir_lowering=False, debug=False)

    # misc = [padb(16) | tri(128) | bq f32-bits(2) | bk f32-bits(2)]
    d_misc = nc.dram_tensor("misc", [128, SC + 128 + 4], bf16,
                            kind="ExternalInput").ap()
    # wqx = [wq(128) | xq granule 0 cols 0:256] per chunk (+ xq0b rest)
    d_wqx = nc.dram_tensor("wqx", [128, EC, 128 + QB // 2], f8e4,
                           kind="ExternalInput").ap()
    d_wkx = nc.dram_tensor("wkx", [128, EC, 128 + QB // 2], f8e4,
                           kind="ExternalInput").ap()
    d_qxb = nc.dram_tensor("qxb", [128, EC, QB // 2], f8e4,
                           kind="ExternalInput").ap()
    d_kxb = nc.dram_tensor("kxb", [128, EC, QB // 2], f8e4,
                           kind="ExternalInput").ap()
    d_qx = nc.dram_tensor("qx", [NG - 1, 128, EC, QB], f8e4,
                          kind="ExternalInput").ap()
    d_kx = nc.dram_tensor("kx", [NG - 1, 128, EC, QB], f8e4,
                          kind="ExternalInput").ap()
    d_wvb = nc.dram_tensor("wvb", [128, EC * 128 + DV], bf16,
                           kind="ExternalInput").ap()
    d_vx = nc.dram_tensor("vx", [NG, 128, EC, QB], bf16,
                          kind="ExternalInput").ap()
    # out groups of rows: 4,4,4,2,1,1 (small final groups shorten the
    # post-row15 DMA tail); partition-major [128, row, DV+1] layout keeps
    # each group's per-partition bytes contiguous in HBM
    d_out = nc.dram_tensor("out", [128, SC, DV + 1], bf16,
                           kind="ExternalOutput").ap()

    with tile.TileContext(nc) as tc, ExitStack() as ctx:
        consts = ctx.enter_context(tc.tile_pool(name="consts", bufs=1))
        xq_p = ctx.enter_context(tc.tile_pool(name="xq", bufs=NG - 1))
        xk_p = ctx.enter_context(tc.tile_pool(name="xk", bufs=NG - 1))
        xv_p = ctx.enter_context(tc.tile_pool(name="xv", bufs=NG))
        proj_p = ctx.enter_context(tc.tile_pool(name="proj", bufs=1))
        attn_p = ctx.enter_context(tc.tile_pool(name="attn", bufs=1))
        out_p = ctx.enter_context(tc.tile_pool(name="outp", bufs=6))
        # PSUM budget: proj/vnat 2 banks + scores 4 + AV 2 = 8
        ps_main = ctx.enter_context(tc.tile_pool(name="ps_main", bufs=2, space="PSUM"))
        ps_sc = ctx.enter_context(tc.tile_pool(name="ps_sc", bufs=2, space="PSUM"))
        ps_av = ctx.enter_context(tc.tile_pool(name="ps_av", bufs=2, space="PSUM"))

        # ---- input DMA issue: ONE ring (sync/SP), arrival-priority order --
        wqx = consts.tile([128, EC, 128 + QB // 2], f8e4, tag="wqx")
        nc.sync.dma_start(wqx[:, :, :], d_wqx)
        wkx = consts.tile([128, EC, 128 + QB // 2], f8e4, tag="wkx")
        nc.sync.dma_start(wkx[:, :, :], d_wkx)
        misc = consts.tile([128, SC + 128 + 4], bf16, tag="misc")
        nc.sync.dma_start(misc[:, :], d_misc)
        padb = misc[:, 0:SC]
        tri = misc[:, SC:SC + 128]
        bq = misc[:, SC + 128:SC + 130].bitcast(f32)
        bk = misc[:, SC + 130:SC + 132].bitcast(f32)
        qxb = consts.tile([128, EC, QB // 2], f8e4, tag="qxb")
        nc.sync.dma_start(qxb[:, :, :], d_qxb)
        kxb = consts.tile([128, EC, QB // 2], f8e4, tag="kxb")
        nc.sync.dma_start(kxb[:, :, :], d_kxb)

        xq = [(wqx, 128)]
        xk = [(wkx, 128)]
        for g in range(1, NG):
            tq = xq_p.tile([128, EC, QB], f8e4, tag="xq", name=f"xq{g}")
            nc.sync.dma_start(tq[:, :, :], d_qx[g - 1])
            xq.append((tq, 0))
            tk = xk_p.tile([128, EC, QB], f8e4, tag="xk", name=f"xk{g}")
            nc.sync.dma_start(tk[:, :, :], d_kx[g - 1])
            xk.append((tk, 0))

        wvb = consts.tile([128, EC * 128 + DV], bf16, tag="wvb")
        nc.sync.dma_start(wvb[:, :], d_wvb)
        bvb = wvb[:, EC * 128:EC * 128 + DV]

        xv = []
        for g in range(NG):
            t = xv_p.tile([128, EC, QB], bf16, tag="xv", name=f"xv{g}")
            nc.sync.dma_start(t[:, :, :], d_vx[g])
            xv.append(t)

        # ---- warmup: exp LUT load + PE HAM ramp during the DMA window ----
        warm = consts.tile([128, QB], bf16, tag="warm")
        nc.gpsimd.memset(warm[:, :], 0.0)
        wo = consts.tile([128, 1], f32, tag="warmo")
        nc.scalar.activation(wo[:, :], warm[:, 0:1], AF.Exp)
        wps = ps_main.tile([128, QB], f32, tag="ps", name="warmps")
        for i in range(7):
            nc.tensor.matmul(wps[:, :], warm[:, 0:128], warm[:, :],
                             start=(i == 0), stop=(i == 6))

        qT = proj_p.tile([128, S], bf16, tag="qT")
        kT = proj_p.tile([128, S], bf16, tag="kT")

        def proj_piece(dst, w, bias, p):
            # dst[:, p*QB:(p+1)*QB] = (64W).T @ x_granule(p) + 64b
            # DoubleRow: K=256 per pass via adjacent chunk pairs (e4m3).
            # Piece 0's granule is split across two transfers (wqx carries
            # cols 0:256, qxb the rest) so the first matmul starts sooner.
            ps = ps_main.tile([128, QB], f32, tag="ps")
            if p == 0:
                xa = wqx if dst is qT else wkx
                xb = qxb if dst is qT else kxb
                H = QB // 2
                for c in range(EC2):
                    nc.tensor.matmul(ps[:, 0:H], w[:, 2 * c:2 * c + 2, 0:128],
                                     xa[:, 2 * c:2 * c + 2, 128:128 + H],
                                     start=(c == 0), stop=(c == EC2 - 1),
                                     perf_mode=DR)
                for c in range(EC2):
                    nc.tensor.matmul(ps[:, H:QB], w[:, 2 * c:2 * c + 2, 0:128],
                                     xb[:, 2 * c:2 * c + 2, :],
                                     start=(c == 0), stop=(c == EC2 - 1),
                                     perf_mode=DR)
            else:
                xt, off = (xq if dst is qT else xk)[p]
                for c in range(EC2):
                    nc.tensor.matmul(ps[:, :], w[:, 2 * c:2 * c + 2, 0:128],
                                     xt[:, 2 * c:2 * c + 2, off:off + QB],
                                     start=(c == 0), stop=(c == EC2 - 1),
                                     perf_mode=DR)
            nc.vector.tensor_scalar(dst[:, p * QB:(p + 1) * QB], ps[:, :],
                                    bias, None, ALU.add)

        attnT = [attn_p.tile([128, S - j * 128], bf16, tag=f"attnT{j}",
                             name=f"attnT{j}")
                 for j in range(SC)]
        vaug = [attn_p.tile([128, DV + 1], bf16, tag=f"vaug{j}",
                            name=f"vaug{j}")
                for j in range(SC)]
        for j in range(SC):
            nc.vector.memset(vaug[j][:, DV:DV + 1], 1.0)

        def scores_win(j, a0, a1):
            # scoresT[j], abs q cols [a0, a1) -> exp -> attnT[j] slice (bf16)
            n = a1 - a0
            ps = ps_sc.tile([128, n], f32, tag="ps_sc")
            for q0 in range(a0, a1, QB):
                m = min(QB, a1 - q0)
                nc.tensor.matmul(ps[:, q0 - a0:q0 - a0 + m],
                                 kT[:, j * 128:(j + 1) * 128],
                                 qT[:, q0:q0 + m], start=True, stop=True)
            nc.scalar.activation(attnT[j][:, a0 - j * 128:a1 - j * 128],
                                 ps[:, :], AF.Exp,
                                 bias=padb[:, j:j + 1],
                                 scale=RSQRT_DQ / (WSCALE * WSCALE))

        def tri_mask(j):
            # in-block causal mask on the diagonal block (keep k <= q)
            nc.vector.tensor_mul(attnT[j][:, 0:128], attnT[j][:, 0:128],
                                 tri)

        def vnat_pair(jp):
            # v natural [keys, DV] for chunks 2jp, 2jp+1: stationary = value
            # seq-slice, moving = Wv chunk; + bias along DV via bvb.
            js = (2 * jp, 2 * jp + 1)
            pss = [ps_main.tile([128, 128], f32, tag="ps", name=f"psv{j}")
                   for j in js]
            for c in range(EC):
                for ji, j in enumerate(js):
                    g, k0 = j // 4, (j % 4) * 128
                    nc.tensor.matmul(pss[ji][:, :], xv[g][:, c, k0:k0 + 128],
                                     wvb[:, c * 128:(c + 1) * 128],
                                     start=(c == 0), stop=(c == EC - 1))
            for ji, j in enumerate(js):
                nc.vector.tensor_add(vaug[j][:, 0:DV], pss[ji][:, :],
                                     bvb)

        OGRP = {}  # row -> (group start, group len)
        for g0, gl in ((0, 4), (4, 4), (8, 4), (12, 2), (14, 1), (15, 1)):
            for i in range(g0, g0 + gl):
                OGRP[i] = (g0, gl)
        ostage = [None]

        def out_row(i, ps):
            # stage output rows and DMA per group: fewer transfers (descgen
            # and DMAHW-lane churn on the sync ring was backpressuring the
            # AV chain through out-tile recycling)
            g0, gl = OGRP[i]
            if i == g0:
                ostage[0] = out_p.tile([128, 4, DV + 1], bf16, tag="ot",
                                       name=f"ost{g0}")
            nc.vector.tensor_copy(ostage[0][:, i - g0, :], ps[:, :])
            if i == g0 + gl - 1:
                nc.sync.dma_start(d_out[:, g0:g0 + gl, :],
                                  ostage[0][:, 0:gl, :])

        def av_row(i):
            ps = ps_av.tile([128, DV + 1], f32, tag="pso")
            for j in range(i + 1):
                nc.tensor.matmul(ps[:, :],
                                 attnT[j][:, (i - j) * 128:(i - j) * 128 + 128],
                                 vaug[j][:, :], start=(j == 0), stop=(j == i))
            out_row(i, ps)

        # ---- interleaved schedule (PE FIFO order == priority order) ----
        # Ordered to match granule arrival (xq g / xk g alternate, xv last).
        # Warm filler matmul groups sit just before the granule-gated pieces
        # so a late DMA can't idle the PE long enough for HAM to re-throttle.
        def filler(name, nmm=2):
            fps = ps_main.tile([128, QB], f32, tag="ps", name=name)
            for i in range(nmm):
                nc.tensor.matmul(fps[:, :], warm[:, 0:128], warm[:, :],
                                 start=(i == 0), stop=(i == nmm - 1))

        proj_piece(qT, wqx, bq, 0)
        filler("fill_k0", 2)
        proj_piece(kT, wkx, bk, 0)
        scores_win(0, 0, 512)
        filler("fill_q1", 2)
        proj_piece(qT, wqx, bq, 1)
        scores_win(0, 512, 1024)
        scores_win(1, 128, 1024)
        filler("fill_k1", 2)
        proj_piece(kT, wkx, bk, 1)
        scores_win(2, 256, 1024)
        scores_win(3, 384, 1024)
        filler("fill_q2", 2)
        proj_piece(qT, wqx, bq, 2)
        scores_win(4, 512, 1024)
        scores_win(5, 640, 1024)
        filler("fill_k2", 2)
        proj_piece(kT, wkx, bk, 2)
        scores_win(6, 768, 1024)
        scores_win(7, 896, 1024)
        proj_piece(qT, wqx, bq, 3)
        scores_win(0, 1024, 2048)
        tri_mask(0)
        proj_piece(kT, wkx, bk, 3)
        scores_win(1, 1024, 2048)
        tri_mask(1)
        scores_win(2, 1024, 2048)
        tri_mask(2)
        vnat_pair(0)
        scores_win(3, 1024, 2048)
        tri_mask(3)
        scores_win(4, 1024, 2048)
        tri_mask(4)
        vnat_pair(1)
        scores_win(5, 1024, 2048)
        tri_mask(5)
        scores_win(6, 1024, 2048)
        tri_mask(6)
        vnat_pair(2)
        scores_win(7, 1024, 2048)
        tri_mask(7)
        scores_win(8, 1024, 2048)
        tri_mask(8)
        vnat_pair(3)
        av_row(0)
        av_row(1)
        scores_win(9, 1152, 2048)
        tri_mask(9)
        av_row(2)
        av_row(3)
        scores_win(10, 1280, 2048)
        tri_mask(10)
        vnat_pair(4)
        av_row(4)
        av_row(5)
        scores_win(11, 1408, 2048)
        tri_mask(11)
        vnat_pair(5)
        av_row(6)
        av_row(7)
        scores_win(12, 1536, 2048)
        tri_mask(12)
        vnat_pair(6)
        av_row(8)
        av_row(9)
        scores_win(13, 1664, 2048)
        tri_mask(13)
        vnat_pair(7)
        # push the last score windows ahead of remaining AV so the exp
        # chain (ACT) finishes while the PE grinds AV matmuls
        scores_win(14, 1792, 2048)
        tri_mask(14)
        av_row(10)
        scores_win(15, 1920, 2048)
        tri_mask(15)
        av_row(11)
        # rows 13-15: accumulate j<=12 while the last exps are still in
        # flight, so the PE tail after exp15 is tiny.
        ps13 = ps_av.tile([128, DV + 1], f32, tag="pso", name="ps13")
        for j in range(0, 13):
            nc.tensor.matmul(ps13[:, :],
                             attnT[j][:, (13 - j) * 128:(13 - j) * 128 + 128],
                             vaug[j][:, :], start=(j == 0), stop=False)
        av_row(12)
        ps14 = ps_main.tile([128, DV + 1], f32, tag="ps", name="ps14")
        for j in range(0, 13):
            nc.tensor.matmul(ps14[:, :],
                             attnT[j][:, (14 - j) * 128:(14 - j) * 128 + 128],
                             vaug[j][:, :], start=(j == 0), stop=False)
        ps15 = ps_main.tile([128, DV + 1], f32, tag="ps", name="ps15")
        for j in range(0, 13):
            nc.tensor.matmul(ps15[:, :],
                             attnT[j][:, (15 - j) * 128:(15 - j) * 128 + 128],
                             vaug[j][:, :], start=(j == 0), stop=False)
        for i, psx in ((13, ps13), (14, ps14), (15, ps15)):
            for j in range(13, i + 1):
                nc.tensor.matmul(psx[:, :],
                                 attnT[j][:, (i - j) * 128:(i - j) * 128 + 128],
                                 vaug[j][:, :], start=False, stop=(j == i))
            out_row(i, psx)

    nc.compile()
    return nc


def _granulize(xT, dtype, width=QB):
    # [E, S] -> [S//width, 128, EC, width]: granule g holds all E rows for
    # seq slice [g*width,(g+1)*width), laid out so each partition line is
    # contiguous in HBM.
    return np.ascontiguousarray(
        xT.reshape(EC, 128, S // width, width).transpose(2, 1, 0, 3)
        .astype(dtype))


def _prep_inputs(pad_mask, query, key, value, Wq, bq, Wk, bk, Wv, bv):
    def wprep(w, dtype, scale):
        return np.ascontiguousarray(
            (np.asarray(w, np.float32) * scale).astype(dtype)
            .reshape(EC, 128, 128).transpose(1, 0, 2))

    wq = wprep(Wq, _E4M3, WSCALE)      # [128, EC, 128]
    wk = wprep(Wk, _E4M3, WSCALE)
    wv = wprep(Wv, _BF16, 1.0)
    wvb = np.concatenate(
        [wv.reshape(128, EC * 128),
         np.broadcast_to(np.asarray(bv, np.float32).astype(_BF16),
                         (128, DV))], axis=1)
    tri = np.triu(np.ones((128, 128), np.float32)).astype(_BF16)

    pad_mask = np.asarray(pad_mask)
    query = np.clip(np.asarray(query, np.float32), -15.0, 15.0)
    key = np.clip(np.asarray(key, np.float32), -15.0, 15.0)
    value = np.asarray(value, np.float32)
    bq64 = np.asarray(bq, np.float32).reshape(128, 1) * WSCALE
    bk64 = np.asarray(bk, np.float32).reshape(128, 1) * WSCALE
    in_maps = []
    for b in range(B):
        padb = np.where(pad_mask[b], NEG, np.float32(0.0)).reshape(SC, 128).T
        misc = np.concatenate(
            [padb.astype(_BF16).view(np.uint16), tri.view(np.uint16),
             bq64.view(np.uint16), bk64.view(np.uint16)],
            axis=1).view(_BF16)
        qg = _granulize(query[b].T, _E4M3)
        kg = _granulize(key[b].T, _E4M3)
        H = QB // 2
        in_maps.append({
            "misc": np.ascontiguousarray(misc),
            "wqx": np.ascontiguousarray(
                np.concatenate([wq, qg[0][:, :, 0:H]], axis=2)),
            "wkx": np.ascontiguousarray(
                np.concatenate([wk, kg[0][:, :, 0:H]], axis=2)),
            "qxb": np.ascontiguousarray(qg[0][:, :, H:]),
            "kxb": np.ascontiguousarray(kg[0][:, :, H:]),
            "qx": np.ascontiguousarray(qg[1:]),
            "kx": np.ascontiguousarray(kg[1:]),
            "wvb": np.ascontiguousarray(wvb),
            "vx": _granulize(value[b].T, _BF16),
        })
    return in_maps


def _run(in_maps, trace=False, **kwargs):
    global _prog
    from concourse.bass_utils import run_bass_kernel_spmd
    if _prog is None:
        _prog = _build_program()
    return run_bass_kernel_spmd(_prog, in_maps, list(range(B)), trace=trace,
                                **kwargs)


def _unstage(arr):
    # [128, SC, DV+1] -> [S, DV+1]: seq = row*128 + p
    return np.ascontiguousarray(
        np.transpose(arr, (1, 0, 2)).reshape(S, DV + 1))


def _finish(raw):
    # raw: [B, S, DV+1] f32 (numerator | row-sum); normalize on host
    return np.ascontiguousarray(
        (raw[:, :, :DV] / raw[:, :, DV:DV + 1]).astype(np.float32))


def kernel(pad_mask, query, key, value, Wq, bq, Wk, bk, Wv, bv):
    in_maps = _prep_inputs(pad_mask, query, key, value, Wq, bq, Wk, bk, Wv, bv)
    res = _run(in_maps)
    raw = np.stack([_unstage(np.asarray(res.results[i]["out"]))
                    for i in range(B)])
    return _finish(raw.astype(np.float32))
